# revision 30
# baseline (speedup 1.0000x reference)
import numpy as np
import ml_dtypes
from contextlib import ExitStack

import concourse.bass as bass
import concourse.tile as tile
from concourse import bacc, mybir
from concourse.bass_utils import run_bass_kernel_spmd

BF = ml_dtypes.bfloat16
B, T, D, H, L, V = 8, 512, 768, 12, 6, 8192
HD, F, P = 64, 3072, 128
NT, NK, NF = T // P, D // P, F // P  # 4, 6, 24
NV = V // 512  # 16 lm-head column chunks
NP = H // 2  # 6 head pairs

_CACHE = {}
TRACE = False
DEBUG = False
LAST = {}


def _build_nc():
    nc = bacc.Bacc("TRN2", target_bir_lowering=False)
    dt = mybir.dt
    d_x0 = nc.dram_tensor("x0", [T, D], dt.float32, kind="ExternalInput")
    d_wq = nc.dram_tensor("wq", [L, D, D], dt.bfloat16, kind="ExternalInput")
    d_wk = nc.dram_tensor("wk", [L, D, D], dt.bfloat16, kind="ExternalInput")
    d_wv = nc.dram_tensor("wv", [L, D, D], dt.bfloat16, kind="ExternalInput")
    d_wo = nc.dram_tensor("wo", [L, D, D], dt.bfloat16, kind="ExternalInput")
    d_w1 = nc.dram_tensor("w1", [L, D, F], dt.bfloat16, kind="ExternalInput")
    d_w2 = nc.dram_tensor("w2", [L, F, D], dt.bfloat16, kind="ExternalInput")
    d_wlm = nc.dram_tensor("wlm", [D, V], dt.bfloat16, kind="ExternalInput")
    d_tri = nc.dram_tensor("tri", [P, P], dt.bfloat16, kind="ExternalInput")
    d_sel = nc.dram_tensor("sel64", [P, P], dt.float32, kind="ExternalInput")
    d_id = nc.dram_tensor("ident", [P, P], dt.float32, kind="ExternalInput")
    d_out = nc.dram_tensor("logits", [T, V], dt.float32, kind="ExternalOutput")
    dbg = {}
    if DEBUG:
        for nm, shape, ddt in (
                ("dbg_qt", [P, NK * T], dt.bfloat16),
                ("dbg_kt", [P, NK * T], dt.bfloat16),
                ("dbg_den", [P, 3 * T], dt.float32),
                ("dbg_attn", [P, NK * T], dt.bfloat16),
                ("dbg_x1", [P, NT * D], dt.float32),
                ("dbg_h2T", [P, NK * T], dt.bfloat16)):
            dbg[nm] = nc.dram_tensor(nm, shape, ddt, kind="ExternalOutput")

    with tile.TileContext(nc) as tc, ExitStack() as ctx:
        _emit(ctx, tc, nc, dt, d_x0, d_wq, d_wk, d_wv, d_wo, d_w1, d_w2,
              d_wlm, d_tri, d_sel, d_id, d_out, dbg)
    nc.compile()
    return nc


def _emit(ctx, tc, nc, dt, d_x0, d_wq, d_wk, d_wv, d_wo, d_w1, d_w2,
          d_wlm, d_tri, d_sel, d_id, d_out, dbg={}):
    ts = bass.ts
    EX = mybir.ActivationFunctionType.Exp
    RL = mybir.ActivationFunctionType.Relu
    CP = mybir.ActivationFunctionType.Copy
    SQ = mybir.ActivationFunctionType.Sqrt
    SQR = mybir.ActivationFunctionType.Square
    IDT = mybir.ActivationFunctionType.Identity
    MUL = mybir.AluOpType.mult
    SUB = mybir.AluOpType.subtract
    ADD = mybir.AluOpType.add

    pool = lambda name, bufs, space="SBUF": ctx.enter_context(
        tc.tile_pool(name=name, bufs=bufs, space=space))

    # persistent SBUF
    pers = pool("pers", 1)
    x = pers.tile([P, NT * D], dt.float32, tag="x")          # residual, [t-tile|D]
    vext = pers.tile([P, NT * H * (HD + 1)], dt.bfloat16, tag="vext")
    uT = pers.tile([P, NF * T], dt.bfloat16, tag="uT")
    tri = pers.tile([P, P], dt.bfloat16, tag="tri")
    ident = pers.tile([P, P], dt.float32, tag="ident")
    sel64 = pers.tile([P, P], dt.float32, tag="sel64")
    # softmax denominators: head h lives at partition 32*(h%4), col block
    # 512*(h//4) -- keeps every partition base 32-aligned for the verifier.
    den = pers.tile([P, 3 * T], dt.float32, tag="den")
    nc.gpsimd.memset(den[:], 1.0)
    eps = pers.tile([P, 1], dt.float32, tag="eps")
    nc.gpsimd.memset(eps[:], 1e-5)
    zero = pers.tile([P, 1], dt.float32, tag="zero")
    nc.gpsimd.memset(zero[:], 0.0)
    nc.const_aps.aps[(dt.float32, 0.0)] = zero[:]

    nc.sync.dma_start(tri[:], d_tri[:, :])
    nc.sync.dma_start(ident[:], d_id[:, :])
    nc.sync.dma_start(sel64[:], d_sel[:, :])
    nc.gpsimd.memset(vext[:], 1.0)
    for t in range(NT):
        nc.sync.dma_start(x[:, ts(t, D)], d_x0[ts(t, P), :])

    # pools
    tposed = pool("tposed", 2)       # hT / attn_n / h2T (rotating)
    qkt = pool("qkt", 1)
    cpool = pool("cpool", 4)
    stats = pool("stats", 2)
    probs_p = pool("probs", 8)
    w_qkvo = pool("w_qkvo", 6)
    w1_p = pool("w1p", 7)
    # t-first FFN2 keeps all 24 w2 k-tiles resident at once
    w2_p = pool("w2p", 24)
    wlm_p = pool("wlmp", 6)
    lout_p = pool("lout", 2)
    psum = pool("psum", 6, "PSUM")
    psum_a = pool("psumA", 2, "PSUM")   # attnV accumulators only

    def ln_stats(t):
        """bn_stats/aggr, rstd = 1/sqrt(var+eps) via ACT Sqrt + DVE recip."""
        xt = x[:, ts(t, D)]
        # NB: bn_aggr's variance combine is only exact for equal group sizes
        s6 = stats.tile([P, 12], dt.float32, tag="s6")
        nc.vector.bn_stats(s6[:, 0:6], xt[:, 0:384])
        nc.vector.bn_stats(s6[:, 6:12], xt[:, 384:768])
        mv = stats.tile([P, 2], dt.float32, tag="mv")
        nc.vector.bn_aggr(mv[:], s6[:])
        sd = stats.tile([P, 1], dt.float32, tag="sd")
        nc.scalar.activation(sd[:], mv[:, 1:2], SQ, bias=eps[:, 0:1])
        rstd = stats.tile([P, 1], dt.float32, tag="rstd")
        nc.vector.reciprocal(rstd[:], sd[:])
        c = cpool.tile([P, D], dt.float32, tag="c")
        nc.vector.tensor_scalar(
            out=c[:], in0=xt, scalar1=mv[:, 0:1], scalar2=rstd[:, 0:1],
            op0=SUB, op1=MUL)
        return c

    def fused_residual_ln(t, r5, r2):
        """x += r (free per-row sums via accum_out) then LN via one ACT
        Square pass: var = (S2 - S1^2/D)/D.  Chain is hidden under the next
        t-tile's matmuls; only the last t's chain is exposed."""
        xt = x[:, ts(t, D)]
        s1a = stats.tile([P, 1], dt.float32, tag="s1a")
        s1b = stats.tile([P, 1], dt.float32, tag="s1b")
        nc.vector.scalar_tensor_tensor(
            out=xt[:, 0:512], in0=xt[:, 0:512], scalar=1.0, in1=r5[:],
            op0=MUL, op1=ADD, accum_out=s1a[:])
        nc.vector.scalar_tensor_tensor(
            out=xt[:, 512:768], in0=xt[:, 512:768], scalar=1.0, in1=r2[:],
            op0=MUL, op1=ADD, accum_out=s1b[:])
        sq = cpool.tile([P, D], dt.float32, tag="c", name="sq")
        s2 = stats.tile([P, 1], dt.float32, tag="s2")
        nc.scalar.activation(sq[:], xt, SQR, accum_out=s2[:])
        s1 = stats.tile([P, 1], dt.float32, tag="s1")
        nc.vector.tensor_add(s1[:], s1a[:], s1b[:])
        qv = stats.tile([P, 1], dt.float32, tag="qv")
        nc.vector.tensor_mul(qv[:], s1[:], s1[:])
        nc.vector.scalar_tensor_tensor(
            out=qv[:], in0=qv[:], scalar=-1.0 / D, in1=s2[:],
            op0=MUL, op1=ADD)
        sd = stats.tile([P, 1], dt.float32, tag="sd")
        nc.scalar.activation(sd[:], qv[:], SQ, bias=eps[:, 0:1], scale=1.0 / D)
        rstd = stats.tile([P, 1], dt.float32, tag="rstd")
        nc.vector.reciprocal(rstd[:], sd[:])
        nmr = stats.tile([P, 1], dt.float32, tag="nmr")
        nc.vector.scalar_tensor_tensor(
            out=nmr[:], in0=s1[:], scalar=-1.0 / D, in1=rstd[:],
            op0=MUL, op1=MUL)
        c = cpool.tile([P, D], dt.float32, tag="c")
        nc.scalar.activation(c[:, 0:384], xt[:, 0:384], IDT,
                             bias=nmr[:, 0:1], scale=rstd[:, 0:1])
        nc.vector.tensor_scalar(
            out=c[:, 384:768], in0=xt[:, 384:768], scalar1=rstd[:, 0:1],
            scalar2=nmr[:, 0:1], op0=MUL, op1=ADD)
        return c

    def ln_transpose(t, c, hdst):
        """transpose c into hdst (k-major [P, NK*T]) columns for t-tile."""
        hv = hdst.rearrange("p (k u) -> p k u", u=T)
        for half in range(2):
            ps = psum.tile([P, 384], dt.float32, tag="ps", name="tp")
            for kk in range(3):
                nc.tensor.transpose(ps[:, ts(kk, P)],
                                    c[:, ts(3 * half + kk, P)], ident[:])
            dstv = hv[:, 3 * half:3 * half + 3, 128 * t:128 * t + P]
            srcv = ps.rearrange("p (k u) -> p k u", u=P)
            if half == 0:
                nc.scalar.activation(dstv, srcv, CP)
            else:
                nc.vector.tensor_copy(dstv, srcv)

    # ---- initial LN1 of layer 0
    hT = tposed.tile([P, NK * T], dt.bfloat16, tag="tposed")
    cs = [ln_stats(t) for t in range(NT)]
    for t in range(NT):
        ln_transpose(t, cs[t], hT)

    def v_tile(t, wv_sb):
        for n in range(2):
            ps = psum.tile([P, 384], dt.float32, tag="ps", name="mmv")
            for k in range(NK):
                nc.tensor.matmul(ps[:], hT[:, 512 * k + 128 * t:512 * k + 128 * t + P],
                                 wv_sb[k][:, ts(n, 384)],
                                 start=(k == 0), stop=(k == NK - 1))
            dst = vext[:, 780 * t + 390 * n:780 * t + 390 * n + 390]
            dst = dst.rearrange("p (h c) -> p h c", c=HD + 1)[:, :, 0:HD]
            nc.vector.tensor_copy(dst, ps.rearrange("p (h c) -> p h c", c=HD))

    for l in range(L):
        # ---- load attention weights (wv first: v_tiles run at layer start)
        wq_sb, wk_sb, wv_sb, wo_sb = [], [], [], []
        for k in range(NK):
            for tg, lst, dram in (("wv", wv_sb, d_wv), ("wq", wq_sb, d_wq),
                                  ("wk", wk_sb, d_wk), ("wo", wo_sb, d_wo)):
                wt = w_qkvo.tile([P, D], dt.bfloat16, tag=tg, name=tg)
                nc.sync.dma_start(wt[:], dram[l, ts(k, P), :])
                lst.append(wt)
        # prefetch all FFN2 weights now: DMA streams during attention+FFN1
        w2_sb = []
        for k in range(NF):
            w2t = w2_p.tile([P, D], dt.bfloat16, tag="w2")
            nc.sync.dma_start(w2t[:], d_w2[l, ts(k, P), :])
            w2_sb.append(w2t)

        # ---- V tiles first: dense PE ramp at layer start
        for t in range(NT):
            v_tile(t, wv_sb)

        qt = qkt.tile([P, NK * T], dt.bfloat16, tag="qt")
        kt = qkt.tile([P, NK * T], dt.bfloat16, tag="kt")
        attn_n = tposed.tile([P, NK * T], dt.bfloat16, tag="tposed")

        def emit_qkt(ko):
            for src, dst, wsb in ((0, qt, wq_sb), (1, kt, wk_sb)):
                ps = psum.tile([P, T], dt.float32, tag="ps", name="mm")
                for k in range(NK):
                    nc.tensor.matmul(ps[:], wsb[k][:, ts(ko, P)], hT[:, ts(k, T)],
                                     start=(k == 0), stop=(k == NK - 1))
                nc.vector.tensor_copy(dst[:, ts(ko, T)], ps[:])

        def attn_block(m):
            """scores+exp+mask+attnV for head pair m; normalize per block."""
            co = 512 * m
            aps = [psum_a.tile([HD + 1, T], dt.float32, tag="aps", name=f"at{i}")
                   for i in range(2)]
            prs = {}
            # phase 1: all scores + exp + (gpsimd mask issued early)
            for j in range(NT):
                nj = T - 128 * j
                for i in range(2):
                    po = 64 * i
                    sp = psum.tile([P, T], dt.float32, tag="ps", name="sc")[:, :nj]
                    nc.tensor.matmul(
                        sp, kt[po:po + HD, co + 128 * j:co + 128 * j + P],
                        qt[po:po + HD, co + 128 * j:co + T],
                        start=True, stop=True)
                    pr = probs_p.tile([P, T], dt.bfloat16, tag="pr", name="pr")[:, :nj]
                    nc.scalar.activation(pr, sp, EX)
                    nc.gpsimd.tensor_mul(pr[:, 0:P], pr[:, 0:P], tri[:])
                    prs[(j, i)] = pr
            # phase 2: unmasked columns (only exp dependency)
            for j in range(NT - 1):
                nj = T - 128 * j
                for i in range(2):
                    h = 2 * m + i
                    vsl = vext[:, 780 * j + 65 * h:780 * j + 65 * h + HD + 1]
                    nc.tensor.matmul(aps[i][:, 128 * (j + 1):T], vsl,
                                     prs[(j, i)][:, P:nj],
                                     start=(j == 0), stop=False)
            # phase 3: masked diagonal blocks
            for j in range(NT):
                for i in range(2):
                    h = 2 * m + i
                    vsl = vext[:, 780 * j + 65 * h:780 * j + 65 * h + HD + 1]
                    nc.tensor.matmul(aps[i][:, 128 * j:128 * j + P], vsl,
                                     prs[(j, i)][:, 0:P],
                                     start=False, stop=(j == NT - 1))
            # den rows to SBUF (ACT); 1/den once per pair-block as a full
            # 128-partition in-place op (the custom DVE recip is only
            # correct at partition base 0)
            for i in range(2):
                h = 2 * m + i
                dp = 32 * (h % 4)
                nc.scalar.activation(den[dp:dp + 1, ts(h // 4, T)],
                                     aps[i][HD:HD + 1, :], CP)
            if m % 2 == 1:
                b = m // 2
                nc.vector.reciprocal_approx_fast(den[:, ts(b, T)],
                                                 den[:, ts(b, T)])
            for i in range(2):
                nc.vector.tensor_copy(attn_n[64 * i:64 * i + HD, ts(m, T)],
                                      aps[i][0:HD, :])

        def attn_norm(m):
            """broadcast 1/den across partitions + scale pair m's attn_n."""
            base = 64 * (m % 2)
            bp = psum.tile([P, T], dt.float32, tag="ps", name="bc")
            nc.tensor.matmul(bp[:], sel64[base:base + HD, :],
                             den[base:base + HD, ts(m // 2, T)],
                             start=True, stop=True)
            blk = attn_n[:, ts(m, T)]
            nc.vector.tensor_mul(blk, blk, bp[:])

        # ---- pipelined projections + attention: qt/kt casts run two pairs
        # ahead of their scores; normalize lags one pair behind
        emit_qkt(0)
        emit_qkt(1)
        for m in range(NP):
            if m + 2 < NP:
                emit_qkt(m + 2)
            attn_block(m)
            if m >= 1:
                attn_norm(m - 1)
        attn_norm(NP - 1)
        if dbg and l == 0:
            nc.sync.dma_start(dbg["dbg_qt"][:, :], qt[:])
            nc.sync.dma_start(dbg["dbg_kt"][:, :], kt[:])
            nc.sync.dma_start(dbg["dbg_den"][:, :], den[:])
            nc.sync.dma_start(dbg["dbg_attn"][:, :], attn_n[:])

        # ---- out-projection + residual + LN2, staggered per t
        h2T = tposed.tile([P, NK * T], dt.bfloat16, tag="tposed")
        cs = []
        for t in range(NT):
            r5 = psum.tile([P, 512], dt.float32, tag="ps", name=f"ra{t}")
            r2 = psum.tile([P, 256], dt.float32, tag="ps", name=f"rb{t}")
            for k in range(NK):
                st = attn_n[:, 512 * k + 128 * t:512 * k + 128 * t + P]
                nc.tensor.matmul(r5[:], st, wo_sb[k][:, 0:512],
                                 start=(k == 0), stop=(k == NK - 1))
                nc.tensor.matmul(r2[:], st, wo_sb[k][:, 512:768],
                                 start=(k == 0), stop=(k == NK - 1))
            cs.append(fused_residual_ln(t, r5, r2))
        for t in range(NT):
            ln_transpose(t, cs[t], h2T)
        if dbg and l == 0:
            nc.sync.dma_start(dbg["dbg_x1"][:, :], x[:])
            nc.sync.dma_start(dbg["dbg_h2T"][:, :], h2T[:])

        # ---- FFN1: uT[f, t] = relu(W1^T @ h2T)
        for g in range(2):
            w1_sb = []
            for k in range(NK):
                wt = w1_p.tile([P, F // 2], dt.bfloat16, tag="w1")
                nc.sync.dma_start(wt[:], d_w1[l, ts(k, P), ts(g, F // 2)])
                w1_sb.append(wt)
            for fl in range(NF // 2):
                f = NF // 2 * g + fl
                ps = psum.tile([P, T], dt.float32, tag="ps", name="mm")
                for k in range(NK):
                    nc.tensor.matmul(ps[:], w1_sb[k][:, ts(fl, P)], h2T[:, ts(k, T)],
                                     start=(k == 0), stop=(k == NK - 1))
                if f % 2 == 0:
                    nc.scalar.activation(uT[:, ts(f, T)], ps[:], RL)
                else:
                    nc.vector.tensor_scalar_max(uT[:, ts(f, T)], ps[:], 0.0)

        # ---- FFN2 + residual + next LN, staggered per t
        hT2 = tposed.tile([P, NK * T], dt.bfloat16, tag="tposed")
        cs = []
        for t in range(NT):
            r5 = psum.tile([P, 512], dt.float32, tag="ps", name=f"fa{t}")
            r2 = psum.tile([P, 256], dt.float32, tag="ps", name=f"fb{t}")
            for k in range(NF):
                st = uT[:, 512 * k + 128 * t:512 * k + 128 * t + P]
                nc.tensor.matmul(r5[:], st, w2_sb[k][:, 0:512],
                                 start=(k == 0), stop=(k == NF - 1))
                nc.tensor.matmul(r2[:], st, w2_sb[k][:, 512:768],
                                 start=(k == 0), stop=(k == NF - 1))
            cs.append(fused_residual_ln(t, r5, r2))
        for t in range(NT):
            ln_transpose(t, cs[t], hT2)
        hT = hT2

    # ---- LM head (hT now holds final-LN transposed activations)
    for nv in range(NV):
        wlm_sb = []
        for k in range(NK):
            wt = wlm_p.tile([P, 512], dt.bfloat16, tag="wlm")
            nc.sync.dma_start(wt[:], d_wlm[ts(k, P), ts(nv, 512)])
            wlm_sb.append(wt)
        for t in range(NT):
            ps = psum.tile([P, 512], dt.float32, tag="ps", name="lm")
            for k in range(NK):
                nc.tensor.matmul(ps[:], hT[:, 512 * k + 128 * t:512 * k + 128 * t + P],
                                 wlm_sb[k][:], start=(k == 0), stop=(k == NK - 1))
            lo = lout_p.tile([P, 512], dt.float32, tag="lo")
            if t % 2 == 0:
                nc.scalar.activation(lo[:], ps[:], CP)
            else:
                nc.vector.tensor_copy(lo[:], ps[:])
            nc.sync.dma_start(d_out[ts(t, P), ts(nv, 512)], lo[:])


def kernel(**inputs):
    inp = {k: np.asarray(v) for k, v in inputs.items()}
    idx = inp["idx"].astype(np.int64)
    x0_all = (inp["tok_emb"][idx] + inp["pos_emb"][None, :, :]).astype(np.float32)

    g1 = inp["ln1_g"][:, :, None]
    g2 = inp["ln2_g"][:, :, None]
    wq = (g1 * inp["Wq"] * (HD ** -0.5)).astype(BF)
    wk = (g1 * inp["Wk"]).astype(BF)
    wv = (g1 * inp["Wv"]).astype(BF)
    wo = inp["Wo"].astype(BF)
    w1 = (g2 * inp["W1"]).astype(BF)
    w2 = inp["W2"].astype(BF)
    wlm = (inp["lnf_g"][:, None] * inp["Wlm"]).astype(BF)

    tri = (np.arange(P)[:, None] <= np.arange(P)[None, :]).astype(BF)
    sel64 = np.zeros((P, P), dtype=np.float32)
    sel64[0, 0:64] = 1
    sel64[32, 64:128] = 1
    sel64[64, 0:64] = 1
    sel64[96, 64:128] = 1
    ident = np.eye(P, dtype=np.float32)

    if "nc" not in _CACHE:
        _CACHE["nc"] = _build_nc()
    nc = _CACHE["nc"]

    shared = dict(wq=wq, wk=wk, wv=wv, wo=wo, w1=w1, w2=w2, wlm=wlm,
                  tri=tri, sel64=sel64, ident=ident)
    in_maps = [dict(x0=x0_all[b], **shared) for b in range(B)]
    res = run_bass_kernel_spmd(nc, in_maps, list(range(B)), trace=TRACE)
    LAST["res"] = res
    out = np.stack([np.asarray(res.results[b]["logits"]) for b in range(B)])
    return out.astype(np.float32)


# revision 31
# speedup vs baseline: 1.0335x; 1.0335x over previous
import numpy as np
import ml_dtypes
from contextlib import ExitStack

import concourse.bass as bass
import concourse.tile as tile
from concourse import bacc, mybir
from concourse.bass_utils import run_bass_kernel_spmd

BF = ml_dtypes.bfloat16
B, T, D, H, L, V = 8, 512, 768, 12, 6, 8192
HD, F, P = 64, 3072, 128
NT, NK, NF = T // P, D // P, F // P  # 4, 6, 24
NV = V // 512  # 16 lm-head column chunks
NP = H // 2  # 6 head pairs

_CACHE = {}
TRACE = False
DEBUG = False
LAST = {}


def _build_nc():
    nc = bacc.Bacc("TRN2", target_bir_lowering=False)
    dt = mybir.dt
    d_x0 = nc.dram_tensor("x0", [T, D], dt.float32, kind="ExternalInput")
    d_wq = nc.dram_tensor("wq", [L, D, D], dt.bfloat16, kind="ExternalInput")
    d_wk = nc.dram_tensor("wk", [L, D, D], dt.bfloat16, kind="ExternalInput")
    d_wv = nc.dram_tensor("wv", [L, D, D], dt.bfloat16, kind="ExternalInput")
    d_wo = nc.dram_tensor("wo", [L, D, D], dt.bfloat16, kind="ExternalInput")
    d_w1 = nc.dram_tensor("w1", [L, D, F], dt.bfloat16, kind="ExternalInput")
    d_w2 = nc.dram_tensor("w2", [L, F, D], dt.bfloat16, kind="ExternalInput")
    d_wlm = nc.dram_tensor("wlm", [D, V], dt.bfloat16, kind="ExternalInput")
    d_tri = nc.dram_tensor("tri", [P, P], dt.bfloat16, kind="ExternalInput")
    d_sel = nc.dram_tensor("sel64", [P, P], dt.float32, kind="ExternalInput")
    d_id = nc.dram_tensor("ident", [P, P], dt.float32, kind="ExternalInput")
    d_out = nc.dram_tensor("logits", [T, V], dt.float32, kind="ExternalOutput")
    dbg = {}
    if DEBUG:
        for nm, shape, ddt in (
                ("dbg_qt", [P, NK * T], dt.bfloat16),
                ("dbg_kt", [P, NK * T], dt.bfloat16),
                ("dbg_den", [P, 3 * T], dt.float32),
                ("dbg_attn", [P, NK * T], dt.bfloat16),
                ("dbg_x1", [P, NT * D], dt.float32),
                ("dbg_h2T", [P, NK * T], dt.bfloat16)):
            dbg[nm] = nc.dram_tensor(nm, shape, ddt, kind="ExternalOutput")

    with tile.TileContext(nc) as tc, ExitStack() as ctx:
        _emit(ctx, tc, nc, dt, d_x0, d_wq, d_wk, d_wv, d_wo, d_w1, d_w2,
              d_wlm, d_tri, d_sel, d_id, d_out, dbg)
    nc.compile()
    return nc


def _emit(ctx, tc, nc, dt, d_x0, d_wq, d_wk, d_wv, d_wo, d_w1, d_w2,
          d_wlm, d_tri, d_sel, d_id, d_out, dbg={}):
    ts = bass.ts
    EX = mybir.ActivationFunctionType.Exp
    RL = mybir.ActivationFunctionType.Relu
    CP = mybir.ActivationFunctionType.Copy
    SQ = mybir.ActivationFunctionType.Sqrt
    SQR = mybir.ActivationFunctionType.Square
    IDT = mybir.ActivationFunctionType.Identity
    MUL = mybir.AluOpType.mult
    SUB = mybir.AluOpType.subtract
    ADD = mybir.AluOpType.add

    pool = lambda name, bufs, space="SBUF": ctx.enter_context(
        tc.tile_pool(name=name, bufs=bufs, space=space))

    # persistent SBUF
    pers = pool("pers", 1)
    x = pers.tile([P, NT * D], dt.float32, tag="x")          # residual, [t-tile|D]
    vext = pers.tile([P, NT * H * (HD + 1)], dt.bfloat16, tag="vext")
    uT = pers.tile([P, NF * T], dt.bfloat16, tag="uT")
    tri = pers.tile([P, P], dt.bfloat16, tag="tri")
    ident = pers.tile([P, P], dt.float32, tag="ident")
    sel64 = pers.tile([P, P], dt.float32, tag="sel64")
    # softmax denominators: head h lives at partition 32*(h%4), col block
    # 512*(h//4) -- keeps every partition base 32-aligned for the verifier.
    den = pers.tile([P, 4 * T], dt.float32, tag="den")
    nc.gpsimd.memset(den[:], 1.0)
    eps = pers.tile([P, 1], dt.float32, tag="eps")
    nc.gpsimd.memset(eps[:], 1e-5)
    zero = pers.tile([P, 1], dt.float32, tag="zero")
    nc.gpsimd.memset(zero[:], 0.0)
    nc.const_aps.aps[(dt.float32, 0.0)] = zero[:]

    nc.sync.dma_start(tri[:], d_tri[:, :])
    nc.sync.dma_start(ident[:], d_id[:, :])
    nc.sync.dma_start(sel64[:], d_sel[:, :])
    nc.gpsimd.memset(vext[:], 1.0)
    for t in range(NT):
        nc.sync.dma_start(x[:, ts(t, D)], d_x0[ts(t, P), :])

    # pools
    tposed = pool("tposed", 2)       # hT / attn_n / h2T (rotating)
    qkt = pool("qkt", 1)
    cpool = pool("cpool", 4)
    stats = pool("stats", 2)
    probs_p = pool("probs", 8)
    w_qkvo = pool("w_qkvo", 6)
    w1_p = pool("w1p", 7)
    # t-first FFN2 keeps all 24 w2 k-tiles resident at once
    w2_p = pool("w2p", 24)
    wlm_p = pool("wlmp", 6)
    lout_p = pool("lout", 2)
    psum = pool("psum", 6, "PSUM")
    psum_a = pool("psumA", 2, "PSUM")   # attnV accumulators only

    def ln_stats(t):
        """bn_stats/aggr, rstd = 1/sqrt(var+eps) via ACT Sqrt + DVE recip."""
        xt = x[:, ts(t, D)]
        # NB: bn_aggr's variance combine is only exact for equal group sizes
        s6 = stats.tile([P, 12], dt.float32, tag="s6")
        nc.vector.bn_stats(s6[:, 0:6], xt[:, 0:384])
        nc.vector.bn_stats(s6[:, 6:12], xt[:, 384:768])
        mv = stats.tile([P, 2], dt.float32, tag="mv")
        nc.vector.bn_aggr(mv[:], s6[:])
        sd = stats.tile([P, 1], dt.float32, tag="sd")
        nc.scalar.activation(sd[:], mv[:, 1:2], SQ, bias=eps[:, 0:1])
        rstd = stats.tile([P, 1], dt.float32, tag="rstd")
        nc.vector.reciprocal(rstd[:], sd[:])
        c = cpool.tile([P, D], dt.float32, tag="c")
        nc.vector.tensor_scalar(
            out=c[:], in0=xt, scalar1=mv[:, 0:1], scalar2=rstd[:, 0:1],
            op0=SUB, op1=MUL)
        return c

    def fused_residual_ln(t, r5, r2):
        """x += r (free per-row sums via accum_out) then LN via one ACT
        Square pass: var = (S2 - S1^2/D)/D.  Chain is hidden under the next
        t-tile's matmuls; only the last t's chain is exposed."""
        xt = x[:, ts(t, D)]
        s1a = stats.tile([P, 1], dt.float32, tag="s1a")
        s1b = stats.tile([P, 1], dt.float32, tag="s1b")
        nc.vector.scalar_tensor_tensor(
            out=xt[:, 0:512], in0=xt[:, 0:512], scalar=1.0, in1=r5[:],
            op0=MUL, op1=ADD, accum_out=s1a[:])
        nc.vector.scalar_tensor_tensor(
            out=xt[:, 512:768], in0=xt[:, 512:768], scalar=1.0, in1=r2[:],
            op0=MUL, op1=ADD, accum_out=s1b[:])
        sq = cpool.tile([P, D], dt.float32, tag="c", name="sq")
        s2 = stats.tile([P, 1], dt.float32, tag="s2")
        nc.scalar.activation(sq[:], xt, SQR, accum_out=s2[:])
        s1 = stats.tile([P, 1], dt.float32, tag="s1")
        nc.vector.tensor_add(s1[:], s1a[:], s1b[:])
        qv = stats.tile([P, 1], dt.float32, tag="qv")
        nc.vector.tensor_mul(qv[:], s1[:], s1[:])
        nc.vector.scalar_tensor_tensor(
            out=qv[:], in0=qv[:], scalar=-1.0 / D, in1=s2[:],
            op0=MUL, op1=ADD)
        sd = stats.tile([P, 1], dt.float32, tag="sd")
        nc.scalar.activation(sd[:], qv[:], SQ, bias=eps[:, 0:1], scale=1.0 / D)
        rstd = stats.tile([P, 1], dt.float32, tag="rstd")
        nc.vector.reciprocal(rstd[:], sd[:])
        nmr = stats.tile([P, 1], dt.float32, tag="nmr")
        nc.vector.scalar_tensor_tensor(
            out=nmr[:], in0=s1[:], scalar=-1.0 / D, in1=rstd[:],
            op0=MUL, op1=MUL)
        c = cpool.tile([P, D], dt.float32, tag="c")
        nc.scalar.activation(c[:, 0:384], xt[:, 0:384], IDT,
                             bias=nmr[:, 0:1], scale=rstd[:, 0:1])
        nc.vector.tensor_scalar(
            out=c[:, 384:768], in0=xt[:, 384:768], scalar1=rstd[:, 0:1],
            scalar2=nmr[:, 0:1], op0=MUL, op1=ADD)
        return c

    def ln_transpose(t, c, hdst):
        """transpose c into hdst (k-major [P, NK*T]) columns for t-tile."""
        hv = hdst.rearrange("p (k u) -> p k u", u=T)
        for half in range(2):
            ps = psum.tile([P, 384], dt.float32, tag="ps", name="tp")
            for kk in range(3):
                nc.tensor.transpose(ps[:, ts(kk, P)],
                                    c[:, ts(3 * half + kk, P)], ident[:])
            dstv = hv[:, 3 * half:3 * half + 3, 128 * t:128 * t + P]
            srcv = ps.rearrange("p (k u) -> p k u", u=P)
            if half == 0:
                nc.scalar.activation(dstv, srcv, CP)
            else:
                nc.vector.tensor_copy(dstv, srcv)

    # ---- initial LN1 of layer 0
    hT = tposed.tile([P, NK * T], dt.bfloat16, tag="tposed")
    cs = [ln_stats(t) for t in range(NT)]
    for t in range(NT):
        ln_transpose(t, cs[t], hT)

    def v_tile(t, wv_sb):
        for n in range(2):
            ps = psum.tile([P, 384], dt.float32, tag="ps", name="mmv")
            for k in range(NK):
                nc.tensor.matmul(ps[:], hT[:, 512 * k + 128 * t:512 * k + 128 * t + P],
                                 wv_sb[k][:, ts(n, 384)],
                                 start=(k == 0), stop=(k == NK - 1))
            dst = vext[:, 780 * t + 390 * n:780 * t + 390 * n + 390]
            dst = dst.rearrange("p (h c) -> p h c", c=HD + 1)[:, :, 0:HD]
            nc.vector.tensor_copy(dst, ps.rearrange("p (h c) -> p h c", c=HD))

    for l in range(L):
        # ---- load attention weights (wv first: v_tiles run at layer start)
        wq_sb, wk_sb, wv_sb, wo_sb = [], [], [], []
        for k in range(NK):
            for tg, lst, dram in (("wv", wv_sb, d_wv), ("wq", wq_sb, d_wq),
                                  ("wk", wk_sb, d_wk), ("wo", wo_sb, d_wo)):
                wt = w_qkvo.tile([P, D], dt.bfloat16, tag=tg, name=tg)
                nc.sync.dma_start(wt[:], dram[l, ts(k, P), :])
                lst.append(wt)
        # prefetch all FFN2 weights now: DMA streams during attention+FFN1
        w2_sb = []
        for k in range(NF):
            w2t = w2_p.tile([P, D], dt.bfloat16, tag="w2")
            nc.sync.dma_start(w2t[:], d_w2[l, ts(k, P), :])
            w2_sb.append(w2t)

        # ---- V tiles first: dense PE ramp at layer start
        for t in range(NT):
            v_tile(t, wv_sb)

        qt = qkt.tile([P, NK * T], dt.bfloat16, tag="qt")
        kt = qkt.tile([P, NK * T], dt.bfloat16, tag="kt")
        attn_n = tposed.tile([P, NK * T], dt.bfloat16, tag="tposed")

        def emit_qkt(ko):
            for src, dst, wsb in ((0, qt, wq_sb), (1, kt, wk_sb)):
                ps = psum.tile([P, T], dt.float32, tag="ps", name="mm")
                for k in range(NK):
                    nc.tensor.matmul(ps[:], wsb[k][:, ts(ko, P)], hT[:, ts(k, T)],
                                     start=(k == 0), stop=(k == NK - 1))
                nc.vector.tensor_copy(dst[:, ts(ko, T)], ps[:])

        def attn_block(m):
            """scores+exp+mask+attnV for head pair m; normalize per block."""
            co = 512 * m
            aps = [psum_a.tile([HD + 1, T], dt.float32, tag="aps", name=f"at{i}")
                   for i in range(2)]
            prs = {}
            # phase 1: all scores + exp + (gpsimd mask issued early)
            for j in range(NT):
                nj = T - 128 * j
                for i in range(2):
                    po = 64 * i
                    sp = psum.tile([P, T], dt.float32, tag="ps", name="sc")[:, :nj]
                    nc.tensor.matmul(
                        sp, kt[po:po + HD, co + 128 * j:co + 128 * j + P],
                        qt[po:po + HD, co + 128 * j:co + T],
                        start=True, stop=True)
                    pr = probs_p.tile([P, T], dt.bfloat16, tag="pr", name="pr")[:, :nj]
                    nc.scalar.activation(pr, sp, EX)
                    nc.gpsimd.tensor_mul(pr[:, 0:P], pr[:, 0:P], tri[:])
                    prs[(j, i)] = pr
            # phase 2: unmasked columns (only exp dependency)
            for j in range(NT - 1):
                nj = T - 128 * j
                for i in range(2):
                    h = 2 * m + i
                    vsl = vext[:, 780 * j + 65 * h:780 * j + 65 * h + HD + 1]
                    nc.tensor.matmul(aps[i][:, 128 * (j + 1):T], vsl,
                                     prs[(j, i)][:, P:nj],
                                     start=(j == 0), stop=False)
            # phase 3: masked diagonal blocks
            for j in range(NT):
                for i in range(2):
                    h = 2 * m + i
                    vsl = vext[:, 780 * j + 65 * h:780 * j + 65 * h + HD + 1]
                    nc.tensor.matmul(aps[i][:, 128 * j:128 * j + P], vsl,
                                     prs[(j, i)][:, 0:P],
                                     start=False, stop=(j == NT - 1))
            # den rows to SBUF (ACT); per-pair 1/den as a full 128-partition
            # in-place op (the custom DVE recip is only correct at
            # partition base 0).  Pair m uses den column block m%4.
            b = m % 4
            for i in range(2):
                nc.scalar.activation(den[32 * i:32 * i + 1, ts(b, T)],
                                     aps[i][HD:HD + 1, :], CP)
            nc.vector.reciprocal_approx_fast(den[:, ts(b, T)],
                                             den[:, ts(b, T)])
            for i in range(2):
                nc.vector.tensor_copy(attn_n[64 * i:64 * i + HD, ts(m, T)],
                                      aps[i][0:HD, :])

        def attn_norm(m):
            """broadcast 1/den across partitions + scale pair m's attn_n."""
            bp = psum.tile([P, T], dt.float32, tag="ps", name="bc")
            nc.tensor.matmul(bp[:], sel64[0:HD, :],
                             den[0:HD, ts(m % 4, T)],
                             start=True, stop=True)
            blk = attn_n[:, ts(m, T)]
            nc.vector.tensor_mul(blk, blk, bp[:])

        # ---- pipelined projections + attention: qt/kt casts run two pairs
        # ahead of their scores; normalize lags one pair behind
        emit_qkt(0)
        emit_qkt(1)
        for m in range(NP):
            if m + 2 < NP:
                emit_qkt(m + 2)
            attn_block(m)
            if m >= 2:
                attn_norm(m - 2)
        attn_norm(NP - 2)
        attn_norm(NP - 1)
        if dbg and l == 0:
            nc.sync.dma_start(dbg["dbg_qt"][:, :], qt[:])
            nc.sync.dma_start(dbg["dbg_kt"][:, :], kt[:])
            nc.sync.dma_start(dbg["dbg_den"][:, :], den[:])
            nc.sync.dma_start(dbg["dbg_attn"][:, :], attn_n[:])

        # ---- out-projection + residual + LN2, staggered per t
        h2T = tposed.tile([P, NK * T], dt.bfloat16, tag="tposed")
        cs = []
        for t in range(NT):
            r5 = psum.tile([P, 512], dt.float32, tag="ps", name=f"ra{t}")
            r2 = psum.tile([P, 256], dt.float32, tag="ps", name=f"rb{t}")
            for k in range(NK):
                st = attn_n[:, 512 * k + 128 * t:512 * k + 128 * t + P]
                nc.tensor.matmul(r5[:], st, wo_sb[k][:, 0:512],
                                 start=(k == 0), stop=(k == NK - 1))
                nc.tensor.matmul(r2[:], st, wo_sb[k][:, 512:768],
                                 start=(k == 0), stop=(k == NK - 1))
            cs.append(fused_residual_ln(t, r5, r2))
        for t in range(NT):
            ln_transpose(t, cs[t], h2T)
        if dbg and l == 0:
            nc.sync.dma_start(dbg["dbg_x1"][:, :], x[:])
            nc.sync.dma_start(dbg["dbg_h2T"][:, :], h2T[:])

        # ---- FFN1: uT[f, t] = relu(W1^T @ h2T)
        for g in range(2):
            w1_sb = []
            for k in range(NK):
                wt = w1_p.tile([P, F // 2], dt.bfloat16, tag="w1")
                nc.sync.dma_start(wt[:], d_w1[l, ts(k, P), ts(g, F // 2)])
                w1_sb.append(wt)
            for fl in range(NF // 2):
                f = NF // 2 * g + fl
                ps = psum.tile([P, T], dt.float32, tag="ps", name="mm")
                for k in range(NK):
                    nc.tensor.matmul(ps[:], w1_sb[k][:, ts(fl, P)], h2T[:, ts(k, T)],
                                     start=(k == 0), stop=(k == NK - 1))
                if f % 2 == 0:
                    nc.scalar.activation(uT[:, ts(f, T)], ps[:], RL)
                else:
                    nc.vector.tensor_scalar_max(uT[:, ts(f, T)], ps[:], 0.0)

        # ---- FFN2 + residual + next LN, staggered per t
        hT2 = tposed.tile([P, NK * T], dt.bfloat16, tag="tposed")
        cs = []
        for t in range(NT):
            r5 = psum.tile([P, 512], dt.float32, tag="ps", name=f"fa{t}")
            r2 = psum.tile([P, 256], dt.float32, tag="ps", name=f"fb{t}")
            for k in range(NF):
                st = uT[:, 512 * k + 128 * t:512 * k + 128 * t + P]
                nc.tensor.matmul(r5[:], st, w2_sb[k][:, 0:512],
                                 start=(k == 0), stop=(k == NF - 1))
                nc.tensor.matmul(r2[:], st, w2_sb[k][:, 512:768],
                                 start=(k == 0), stop=(k == NF - 1))
            cs.append(fused_residual_ln(t, r5, r2))
        for t in range(NT):
            ln_transpose(t, cs[t], hT2)
        hT = hT2

    # ---- LM head (hT now holds final-LN transposed activations)
    for nv in range(NV):
        wlm_sb = []
        for k in range(NK):
            wt = wlm_p.tile([P, 512], dt.bfloat16, tag="wlm")
            nc.sync.dma_start(wt[:], d_wlm[ts(k, P), ts(nv, 512)])
            wlm_sb.append(wt)
        for t in range(NT):
            ps = psum.tile([P, 512], dt.float32, tag="ps", name="lm")
            for k in range(NK):
                nc.tensor.matmul(ps[:], hT[:, 512 * k + 128 * t:512 * k + 128 * t + P],
                                 wlm_sb[k][:], start=(k == 0), stop=(k == NK - 1))
            lo = lout_p.tile([P, 512], dt.float32, tag="lo")
            if t % 2 == 0:
                nc.scalar.activation(lo[:], ps[:], CP)
            else:
                nc.vector.tensor_copy(lo[:], ps[:])
            nc.sync.dma_start(d_out[ts(t, P), ts(nv, 512)], lo[:])


def kernel(**inputs):
    inp = {k: np.asarray(v) for k, v in inputs.items()}
    idx = inp["idx"].astype(np.int64)
    x0_all = (inp["tok_emb"][idx] + inp["pos_emb"][None, :, :]).astype(np.float32)

    g1 = inp["ln1_g"][:, :, None]
    g2 = inp["ln2_g"][:, :, None]
    wq = (g1 * inp["Wq"] * (HD ** -0.5)).astype(BF)
    wk = (g1 * inp["Wk"]).astype(BF)
    wv = (g1 * inp["Wv"]).astype(BF)
    wo = inp["Wo"].astype(BF)
    w1 = (g2 * inp["W1"]).astype(BF)
    w2 = inp["W2"].astype(BF)
    wlm = (inp["lnf_g"][:, None] * inp["Wlm"]).astype(BF)

    tri = (np.arange(P)[:, None] <= np.arange(P)[None, :]).astype(BF)
    sel64 = np.zeros((P, P), dtype=np.float32)
    sel64[0, 0:64] = 1
    sel64[32, 64:128] = 1
    sel64[64, 0:64] = 1
    sel64[96, 64:128] = 1
    ident = np.eye(P, dtype=np.float32)

    if "nc" not in _CACHE:
        _CACHE["nc"] = _build_nc()
    nc = _CACHE["nc"]

    shared = dict(wq=wq, wk=wk, wv=wv, wo=wo, w1=w1, w2=w2, wlm=wlm,
                  tri=tri, sel64=sel64, ident=ident)
    in_maps = [dict(x0=x0_all[b], **shared) for b in range(B)]
    res = run_bass_kernel_spmd(nc, in_maps, list(range(B)), trace=TRACE)
    LAST["res"] = res
    out = np.stack([np.asarray(res.results[b]["logits"]) for b in range(B)])
    return out.astype(np.float32)
